# revision 18
# baseline (speedup 1.0000x reference)
"""CRF loss kernel for Trainium2 (8 NeuronCores, data-parallel over batch).

Per-core design (batch shard of 64 rows = 32768 positions, laid out as
[128 partitions x 256 columns], position = p*256 + k, i.e. partition p
holds half of sequence p//2):

  - ONE fused f32r matmul per 128-position column k:
      stationary  Hm_k            [128 pos, 128 tag]  (one-hot of cur tags)
      moving      [E_k | Hm_{k-1}][128 pos, 256]
    accumulated over all k into a single PSUM tile [128, 256]:
      cols 0:128   = sum_k Hm_k^T E_k      (diag = per-tag emission sums)
      cols 128:256 = sum_k Hm_k^T Hm_{k-1} (pair-count matrix, cur x prev)
    f32r with moving free dim 256 runs at full PE rate, so no bf16
    hi/lo split is needed anywhere.
  - One-hots are built by is_equal(iota, tag-column) with masked tags
    folded out of range (tag + 128*(1-m), host-precomputed); builds are
    split DVE/GpSimd to keep both under the DMA roofline.
  - Because consecutive positions sit in consecutive columns of one
    partition, the prev-tag one-hot for column k IS the cur-tag one-hot
    of column k-1 — each one-hot is built once and used twice. The
    k==0 column (sequence starts / partition boundary) uses a
    host-precomputed prev-tag column folded by the pair mask.
  - Epilogue: psum * [identity | transitions^T] row-reduced + mask count
    into a [128, 4] result per core; the cross-partition and cross-core
    sums plus the final division happen on host.
"""
import sys
import json

for p in ('/opt/trn_rl_repo', '/opt/trn_rl_repo/concourse'):
    if p not in sys.path:
        sys.path.insert(0, p)

import numpy as np

B, S, T = 512, 512, 128
NCORES = 8
BSH = B // NCORES              # 64 batch rows per core
NPOS = BSH * S                 # 32768 positions per core
P = 128                        # SBUF partitions
CPT = NPOS // P                # 256 position-columns per partition
J = 8                          # columns per DMA group
G = CPT // J                   # 32 groups


def _split_waits_json(bir_bytes: bytes, max_waits: int = 1) -> bytes:
    """This walrus build accepts at most ONE sync-wait per instruction;
    hoist extra waits onto single-wait NoOps inserted before the inst."""
    d = json.loads(bir_bytes)
    ctr = 0
    for f in d['functions']:
        for blk in f['blocks']:
            insts = blk.get('instructions')
            if not insts:
                continue
            out = []
            changed = False
            for ins in insts:
                si = ins.get('sync_info')
                if si and len(si.get('on_wait') or []) > max_waits:
                    waits = si['on_wait']
                    for w in waits[:-max_waits]:
                        ctr += 1
                        nop = {'engine': ins['engine'], 'ins': [], 'outs': [],
                               'name': f'wsplit-{ctr}', 'opcode': 'NoOp',
                               'sync_info': {'on_wait': [w], 'on_update': []}}
                        if 'debug' in ins:
                            nop['debug'] = ins['debug']
                        out.append(nop)
                    si['on_wait'] = waits[-max_waits:]
                    changed = True
                out.append(ins)
            if changed:
                blk['instructions'] = out
    return json.dumps(d).encode()


_patched = False


def _install_patch(bass_module):
    global _patched
    if _patched:
        return
    _patched = True
    orig = bass_module.Bass.to_json_bytes

    def patched(self):
        return _split_waits_json(orig(self))

    bass_module.Bass.to_json_bytes = patched


def _build():
    import concourse.bass as bass
    import concourse.mybir as mybir
    import concourse.tile as tile
    from concourse.masks import make_identity
    _install_patch(bass)
    f32 = mybir.dt.float32
    f32r = mybir.dt.float32r
    i32 = mybir.dt.int32
    Alu = mybir.AluOpType

    nc = bass.Bass()
    em = nc.dram_tensor('em', [NPOS, T], f32r, kind='ExternalInput')
    mtag = nc.dram_tensor('mtag', [P, CPT + 2 + 2 * T], mybir.dt.uint16,
                          kind='ExternalInput')
    out = nc.dram_tensor('out', [P, 4], f32, kind='ExternalOutput')

    # [p, a, t] view of emissions: column a of partition p = position p*CPT+a
    em_v = em.rearrange("(p a) t -> p a t", p=P)

    # DMA chunks: (start column, width). Tapered tail so the final
    # DMA-dependent matmul burst (and thus the kernel tail) is short.
    CHUNKS = [(i * J, J) for i in range(G - 1)] + \
             [(CPT - J, 4), (CPT - 4, 2), (CPT - 2, 1), (CPT - 1, 1)]

    with tile.TileContext(nc) as tc:
        with tc.tile_pool(name='per', bufs=1) as per, \
             tc.tile_pool(name='stgp', bufs=8) as stgp, \
             tc.tile_pool(name='ps', bufs=1, space='PSUM') as psp:

            # First emissions chunk DMA goes out before anything else.
            tiles = {}
            c0, n0 = CHUNKS[0]
            tiles[0] = stgp.tile([P, J, 2 * T], f32r, tag='stg', name='stg')
            nc.sync.dma_start(out=tiles[0][:, 0:n0, 0:T],
                              in_=em_v[:, c0:c0 + n0, :])

            # ---- constants / small inputs (small DMAs on Act queue) ----
            iota_i = per.tile([P, T], i32)
            nc.gpsimd.iota(iota_i, pattern=[[1, T]], base=0, channel_multiplier=0)
            iota_f = per.tile([P, T], f32)
            nc.vector.tensor_copy(iota_f, iota_i)

            mtag_u = per.tile([P, CPT + 2 + 2 * T], mybir.dt.uint16)
            nc.sync.dma_start(out=mtag_u, in_=mtag[:, :])
            mtag_sb = per.tile([P, CPT + 2], f32)
            nc.vector.tensor_copy(mtag_sb, mtag_u[:, 0:CPT + 2])
            ptag0_sb = mtag_sb[:, CPT:CPT + 1]

            catid = per.tile([P, 2 * T], f32)
            make_identity(nc, catid[:, 0:T])
            nc.vector.tensor_copy(catid[:, T:2 * T],
                                  mtag_u[:, CPT + 2:].bitcast(f32))

            red = per.tile([P, 4], f32)
            nc.vector.memset(red, 0.0)

            hm_last = per.tile([P, T], f32r)
            nc.gpsimd.tensor_scalar(out=hm_last, in0=iota_f,
                                    scalar1=mtag_sb[:, CPT - 1:CPT],
                                    scalar2=None, op0=Alu.is_equal)

            ps = psp.tile([P, 2 * T], f32)

            # ---- main loop ----
            # slot (m, i) cols T:2T holds Hm of column c0_m+i-1; matmuls for
            # chunk m-1 are emitted after chunk m's one-hots so each one-hot
            # serves as moving operand for column c and stationary for c+1.
            for m, (c0, n) in enumerate(CHUNKS):
                if m > 0:
                    tiles[m] = stgp.tile([P, J, 2 * T], f32r, tag='stg', name='stg')
                    nc.sync.dma_start(out=tiles[m][:, 0:n, 0:T],
                                      in_=em_v[:, c0:c0 + n, :])
                stg = tiles[m]
                ndve = max(1, (5 * n) // J)
                for i in range(n):
                    c = c0 + i - 1
                    scal = ptag0_sb if c < 0 else mtag_sb[:, c:c + 1]
                    eng = nc.vector if i < ndve else nc.gpsimd
                    eng.tensor_scalar(out=stg[:, i, T:2 * T], in0=iota_f,
                                      scalar1=scal, scalar2=None,
                                      op0=Alu.is_equal)
                if m > 0:
                    pc0, pn = CHUNKS[m - 1]
                    prev = tiles[m - 1]
                    for i in range(pn):
                        lhsT = prev[:, i + 1, T:2 * T] if i < pn - 1 \
                            else stg[:, 0, T:2 * T]
                        nc.tensor.matmul(ps, lhsT=lhsT, rhs=prev[:, i, :],
                                         start=(pc0 + i == 0), stop=False,
                                         skip_group_check=True)
                tiles.pop(m - 9, None)

            lc0, ln = CHUNKS[-1]
            prev = tiles[len(CHUNKS) - 1]
            for i in range(ln):
                lhsT = prev[:, i + 1, T:2 * T] if i < ln - 1 else hm_last
                nc.tensor.matmul(ps, lhsT=lhsT, rhs=prev[:, i, :],
                                 start=False, stop=(i == ln - 1),
                                 skip_group_check=True)

            # ---- final reductions ----
            # mask count (independent of the matmul chain)
            cnt = per.tile([P, CPT], f32)
            nc.vector.tensor_scalar(out=cnt, in0=mtag_sb[:, 0:CPT], scalar1=float(T),
                                    scalar2=None, op0=Alu.is_lt)
            nc.vector.tensor_reduce(out=red[:, 1:2], in_=cnt,
                                    axis=mybir.AxisListType.X, op=Alu.add)
            # score halves in parallel: emission (diag) on DVE, transition
            # counts * T^T on GpSimd; host adds red cols 0 and 2.
            scr = per.tile([P, 2 * T], f32)
            nc.vector.tensor_mul(scr, ps, catid)
            nc.vector.tensor_reduce(out=red[:, 0:1], in_=scr,
                                    axis=mybir.AxisListType.X, op=Alu.add)
            nc.sync.dma_start(out=out[:, :], in_=red)

    return nc


_nc_cache = None
last_results = None


def kernel(emissions, tags, mask, transitions, _trace=False):
    global _nc_cache, last_results
    from concourse.bass_utils import run_bass_kernel_spmd
    if _nc_cache is None:
        _nc_cache = _build()
    nc = _nc_cache

    em_flat = np.ascontiguousarray(
        np.asarray(emissions).reshape(B * S, T).astype(np.float32, copy=False))
    tg_all = np.asarray(tags).reshape(-1).astype(np.int32)
    mk_all = np.asarray(mask).reshape(-1).astype(np.int32)
    trT = np.ascontiguousarray(np.asarray(transitions).T.astype(np.float32))

    in_maps = []
    podd = np.arange(1, P, 2)
    for c in range(NCORES):
        lo, hi = c * NPOS, (c + 1) * NPOS
        tg2d = tg_all[lo:hi].reshape(P, CPT)
        mk2d = mk_all[lo:hi].reshape(P, CPT)
        mtag2d = np.full((P, CPT + 2 + 2 * T), T, dtype=np.uint16)
        mtag2d[:, 0:CPT] = (tg2d + T * (1 - mk2d)).astype(np.uint16)
        pm = (mk2d[podd, 0] & mk2d[podd - 1, CPT - 1]).astype(bool)
        mtag2d[podd, CPT] = np.where(pm, tg2d[podd - 1, CPT - 1], T).astype(np.uint16)
        mtag2d[:, CPT + 2:] = trT.view(np.uint16)
        in_maps.append({'em': np.ascontiguousarray(em_flat[lo:hi]),
                        'mtag': mtag2d})

    res = run_bass_kernel_spmd(nc, in_maps, core_ids=list(range(NCORES)),
                               trace=_trace)
    last_results = res
    score = cnt = 0.0
    for r in res.results:
        v = np.asarray(r['out'], dtype=np.float64)
        score += float(v[:, 0].sum())
        cnt += float(v[:, 1].sum())
    return np.float32(score / cnt)


# revision 19
# speedup vs baseline: 1.0034x; 1.0034x over previous
"""CRF loss kernel for Trainium2 (8 NeuronCores, data-parallel over batch).

Per-core design (batch shard of 64 rows = 32768 positions, laid out as
[128 partitions x 256 columns], position = p*256 + k, i.e. partition p
holds half of sequence p//2):

  - ONE fused f32r matmul per 128-position column k:
      stationary  Hm_k            [128 pos, 128 tag]  (one-hot of cur tags)
      moving      [E_k | Hm_{k-1}][128 pos, 256]
    accumulated over all k into a single PSUM tile [128, 256]:
      cols 0:128   = sum_k Hm_k^T E_k      (diag = per-tag emission sums)
      cols 128:256 = sum_k Hm_k^T Hm_{k-1} (pair-count matrix, cur x prev)
    f32r with moving free dim 256 runs at full PE rate, so no bf16
    hi/lo split is needed anywhere.
  - One-hots are built by is_equal(iota, tag-column) with masked tags
    folded out of range (tag + 128*(1-m), host-precomputed); builds are
    split DVE/GpSimd to keep both under the DMA roofline.
  - Because consecutive positions sit in consecutive columns of one
    partition, the prev-tag one-hot for column k IS the cur-tag one-hot
    of column k-1 — each one-hot is built once and used twice. The
    k==0 column (sequence starts / partition boundary) uses a
    host-precomputed prev-tag column folded by the pair mask.
  - Epilogue: psum * [identity | transitions^T] row-reduced + mask count
    into a [128, 4] result per core; the cross-partition and cross-core
    sums plus the final division happen on host.
"""
import sys
import json

for p in ('/opt/trn_rl_repo', '/opt/trn_rl_repo/concourse'):
    if p not in sys.path:
        sys.path.insert(0, p)

import numpy as np
import ml_dtypes


def jnp_bf16(a):
    return a.astype(ml_dtypes.bfloat16)

B, S, T = 512, 512, 128
NCORES = 8
BSH = B // NCORES              # 64 batch rows per core
NPOS = BSH * S                 # 32768 positions per core
P = 128                        # SBUF partitions
CPT = NPOS // P                # 256 position-columns per partition
J = 8                          # columns per DMA group
G = CPT // J                   # 32 groups


def _split_waits_json(bir_bytes: bytes, max_waits: int = 1) -> bytes:
    """This walrus build accepts at most ONE sync-wait per instruction;
    hoist extra waits onto single-wait NoOps inserted before the inst."""
    d = json.loads(bir_bytes)
    ctr = 0
    for f in d['functions']:
        for blk in f['blocks']:
            insts = blk.get('instructions')
            if not insts:
                continue
            out = []
            changed = False
            for ins in insts:
                si = ins.get('sync_info')
                if si and len(si.get('on_wait') or []) > max_waits:
                    waits = si['on_wait']
                    for w in waits[:-max_waits]:
                        ctr += 1
                        nop = {'engine': ins['engine'], 'ins': [], 'outs': [],
                               'name': f'wsplit-{ctr}', 'opcode': 'NoOp',
                               'sync_info': {'on_wait': [w], 'on_update': []}}
                        if 'debug' in ins:
                            nop['debug'] = ins['debug']
                        out.append(nop)
                    si['on_wait'] = waits[-max_waits:]
                    changed = True
                out.append(ins)
            if changed:
                blk['instructions'] = out
    return json.dumps(d).encode()


_patched = False


def _install_patch(bass_module):
    global _patched
    if _patched:
        return
    _patched = True
    orig = bass_module.Bass.to_json_bytes

    def patched(self):
        return _split_waits_json(orig(self))

    bass_module.Bass.to_json_bytes = patched


def _build():
    import concourse.bass as bass
    import concourse.mybir as mybir
    import concourse.tile as tile
    from concourse.masks import make_identity
    _install_patch(bass)
    f32 = mybir.dt.float32
    f32r = mybir.dt.float32r
    i32 = mybir.dt.int32
    Alu = mybir.AluOpType

    nc = bass.Bass()
    em = nc.dram_tensor('em', [NPOS, T], f32r, kind='ExternalInput')
    mtag = nc.dram_tensor('mtag', [P, CPT + 2 + 2 * T], mybir.dt.uint8,
                          kind='ExternalInput')
    out = nc.dram_tensor('out', [P, 4], f32, kind='ExternalOutput')

    # [p, a, t] view of emissions: column a of partition p = position p*CPT+a
    em_v = em.rearrange("(p a) t -> p a t", p=P)

    # DMA chunks: (start column, width). Tapered tail so the final
    # DMA-dependent matmul burst (and thus the kernel tail) is short.
    CHUNKS = [(i * J, J) for i in range(G - 1)] + \
             [(CPT - J, 4), (CPT - 4, 2), (CPT - 2, 1), (CPT - 1, 1)]

    with tile.TileContext(nc) as tc:
        with tc.tile_pool(name='per', bufs=1) as per, \
             tc.tile_pool(name='stgp', bufs=8) as stgp, \
             tc.tile_pool(name='ps', bufs=1, space='PSUM') as psp:

            # First emissions chunk DMA goes out before anything else.
            tiles = {}
            c0, n0 = CHUNKS[0]
            tiles[0] = stgp.tile([P, J, 2 * T], f32r, tag='stg', name='stg')
            nc.sync.dma_start(out=tiles[0][:, 0:n0, 0:T],
                              in_=em_v[:, c0:c0 + n0, :])

            # ---- constants / small inputs (small DMAs on Act queue) ----
            iota_i = per.tile([P, T], i32)
            nc.gpsimd.iota(iota_i, pattern=[[1, T]], base=0, channel_multiplier=0)
            iota_f = per.tile([P, T], f32)
            nc.vector.tensor_copy(iota_f, iota_i)

            mtag_u = per.tile([P, CPT + 2 + 2 * T], mybir.dt.uint8)
            nc.sync.dma_start(out=mtag_u, in_=mtag[:, :])
            mtag_sb = per.tile([P, CPT + 2], f32)
            nc.vector.tensor_copy(mtag_sb, mtag_u[:, 0:CPT + 2])
            ptag0_sb = mtag_sb[:, CPT:CPT + 1]

            catid = per.tile([P, 2 * T], f32)
            make_identity(nc, catid[:, 0:T])
            nc.vector.tensor_copy(catid[:, T:2 * T],
                                  mtag_u[:, CPT + 2:].bitcast(mybir.dt.bfloat16))

            red = per.tile([P, 4], f32)
            nc.vector.memset(red, 0.0)

            hm_last = per.tile([P, T], f32r)
            nc.gpsimd.tensor_scalar(out=hm_last, in0=iota_f,
                                    scalar1=mtag_sb[:, CPT - 1:CPT],
                                    scalar2=None, op0=Alu.is_equal)

            ps = psp.tile([P, 2 * T], f32)

            # ---- main loop ----
            # slot (m, i) cols T:2T holds Hm of column c0_m+i-1; matmuls for
            # chunk m-1 are emitted after chunk m's one-hots so each one-hot
            # serves as moving operand for column c and stationary for c+1.
            for m, (c0, n) in enumerate(CHUNKS):
                if m > 0:
                    tiles[m] = stgp.tile([P, J, 2 * T], f32r, tag='stg', name='stg')
                    nc.sync.dma_start(out=tiles[m][:, 0:n, 0:T],
                                      in_=em_v[:, c0:c0 + n, :])
                stg = tiles[m]
                ndve = max(1, (5 * n) // J)
                for i in range(n):
                    c = c0 + i - 1
                    scal = ptag0_sb if c < 0 else mtag_sb[:, c:c + 1]
                    eng = nc.vector if i < ndve else nc.gpsimd
                    eng.tensor_scalar(out=stg[:, i, T:2 * T], in0=iota_f,
                                      scalar1=scal, scalar2=None,
                                      op0=Alu.is_equal)
                if m > 0:
                    pc0, pn = CHUNKS[m - 1]
                    prev = tiles[m - 1]
                    for i in range(pn):
                        lhsT = prev[:, i + 1, T:2 * T] if i < pn - 1 \
                            else stg[:, 0, T:2 * T]
                        nc.tensor.matmul(ps, lhsT=lhsT, rhs=prev[:, i, :],
                                         start=(pc0 + i == 0), stop=False,
                                         skip_group_check=True)
                tiles.pop(m - 9, None)

            lc0, ln = CHUNKS[-1]
            prev = tiles[len(CHUNKS) - 1]
            for i in range(ln):
                lhsT = prev[:, i + 1, T:2 * T] if i < ln - 1 else hm_last
                nc.tensor.matmul(ps, lhsT=lhsT, rhs=prev[:, i, :],
                                 start=False, stop=(i == ln - 1),
                                 skip_group_check=True)

            # ---- final reductions ----
            # mask count (independent of the matmul chain)
            cnt = per.tile([P, CPT], f32)
            nc.vector.tensor_scalar(out=cnt, in0=mtag_sb[:, 0:CPT], scalar1=float(T),
                                    scalar2=None, op0=Alu.is_lt)
            nc.vector.tensor_reduce(out=red[:, 1:2], in_=cnt,
                                    axis=mybir.AxisListType.X, op=Alu.add)
            # score halves in parallel: emission (diag) on DVE, transition
            # counts * T^T on GpSimd; host adds red cols 0 and 2.
            scr = per.tile([P, 2 * T], f32)
            nc.vector.tensor_mul(scr, ps, catid)
            nc.vector.tensor_reduce(out=red[:, 0:1], in_=scr,
                                    axis=mybir.AxisListType.X, op=Alu.add)
            nc.sync.dma_start(out=out[:, :], in_=red)

    return nc


_nc_cache = None
last_results = None


def kernel(emissions, tags, mask, transitions, _trace=False):
    global _nc_cache, last_results
    from concourse.bass_utils import run_bass_kernel_spmd
    if _nc_cache is None:
        _nc_cache = _build()
    nc = _nc_cache

    em_flat = np.ascontiguousarray(
        np.asarray(emissions).reshape(B * S, T).astype(np.float32, copy=False))
    tg_all = np.asarray(tags).reshape(-1).astype(np.int32)
    mk_all = np.asarray(mask).reshape(-1).astype(np.int32)
    trT = np.ascontiguousarray(np.asarray(transitions).T.astype(np.float32))

    in_maps = []
    podd = np.arange(1, P, 2)
    for c in range(NCORES):
        lo, hi = c * NPOS, (c + 1) * NPOS
        tg2d = tg_all[lo:hi].reshape(P, CPT)
        mk2d = mk_all[lo:hi].reshape(P, CPT)
        mtag2d = np.full((P, CPT + 2 + 2 * T), T, dtype=np.uint8)
        mtag2d[:, 0:CPT] = (tg2d + T * (1 - mk2d)).astype(np.uint8)
        pm = (mk2d[podd, 0] & mk2d[podd - 1, CPT - 1]).astype(bool)
        mtag2d[podd, CPT] = np.where(pm, tg2d[podd - 1, CPT - 1], T).astype(np.uint8)
        trT_bf16 = jnp_bf16(trT)
        mtag2d[:, CPT + 2:] = trT_bf16.view(np.uint8).reshape(P, 2 * T)
        in_maps.append({'em': np.ascontiguousarray(em_flat[lo:hi]),
                        'mtag': mtag2d})

    res = run_bass_kernel_spmd(nc, in_maps, core_ids=list(range(NCORES)),
                               trace=_trace)
    last_results = res
    score = cnt = 0.0
    for r in res.results:
        v = np.asarray(r['out'], dtype=np.float64)
        score += float(v[:, 0].sum())
        cnt += float(v[:, 1].sum())
    return np.float32(score / cnt)
